# revision 1
# baseline (speedup 1.0000x reference)
"""Trainium2 Bass kernel for nn_MultiHeadAttention_37477884625313.

Multi-head attention (B=4, C=512, T=1024, H=8, d=64) with the reference's
relative-position terms:
  - score-side rel term lands at 21 corner entries per head
    (j-i >= 1019 with emb_k rows 0-4 on q_i; j-i <= -1021 with emb_k rows
    6-8 on q_{i-1})
  - ctx-side rel term is the textbook +/-4 diagonal band of attn with emb_v

Sharding: 8 cores = 4 batches x 2 query-halves.  Each core computes the full
attention for its (batch, query-half); k/v time axes are host-rotated by -i0
so the SPMD program is identical across cores.  Host concatenates outputs.
"""

import sys

sys.path.insert(0, "/opt/trn_rl_repo")

import numpy as np
import ml_dtypes

import concourse.bass as bass
import concourse.mybir as mybir
import concourse.tile as tile
from concourse import bacc
from concourse.ap import AP
from concourse.masks import make_identity
from concourse.bass_utils import run_bass_kernel_spmd

BF16 = ml_dtypes.bfloat16
P = 128
C = 512
T = 1024
H = 8
D = 64
TQ = 512          # queries per core
IB = 4            # i-blocks per core
NCT = 4           # channel tiles (512 / 128)
W = 136           # band window width (128 + 8)
EXP = mybir.ActivationFunctionType.Exp
ADD = None  # set after import
MUL = None

_CACHE = {}


def build_nc():
    nc = bacc.Bacc("TRN2", target_bir_lowering=False)
    f32, bf = mybir.dt.float32, mybir.dt.bfloat16
    add_op = mybir.AluOpType.add
    mul_op = mybir.AluOpType.mult

    xq = nc.declare_dram_parameter("xq", [P, NCT, TQ], bf, isOutput=False)
    xk = nc.declare_dram_parameter("xk", [P, NCT, T], bf, isOutput=False)
    xv = nc.declare_dram_parameter("xv", [P, NCT, T], bf, isOutput=False)
    wqT = nc.declare_dram_parameter("wqT", [P, NCT, C], bf, isOutput=False)
    wkT = nc.declare_dram_parameter("wkT", [P, NCT, C], bf, isOutput=False)
    wvT = nc.declare_dram_parameter("wvT", [P, NCT, C], bf, isOutput=False)
    woT = nc.declare_dram_parameter("woT", [P, NCT, C], bf, isOutput=False)
    bq8 = nc.declare_dram_parameter("bq8", [P, NCT], f32, isOutput=False)
    evpr = nc.declare_dram_parameter("evpr", [P, NCT, P], bf, isOutput=False)
    oneh = nc.declare_dram_parameter("oneh", [8, NCT, P], f32, isOutput=False)
    cor1d = nc.declare_dram_parameter("cor1d", [P, H, 5], bf, isOutput=False)
    cor2d = nc.declare_dram_parameter("cor2d", [P, H, 3], bf, isOutput=False)
    amask = nc.declare_dram_parameter("amask", [P, IB, P], bf, isOutput=False)
    Out = nc.declare_dram_parameter("Out", [NCT, P, TQ], f32, isOutput=True)

    stage = nc.dram_tensor("stage", [IB, P, H, W], bf)

    with tile.TileContext(nc) as tc:
        with (
            tc.tile_pool(name="persist", bufs=1) as pp,
            tc.tile_pool(name="work", bufs=2) as wp,
            tc.tile_pool(name="epool", bufs=2) as ep,
            tc.tile_pool(name="psA", bufs=2, space="PSUM") as psA,
            tc.tile_pool(name="psB", bufs=2, space="PSUM") as psB,
        ):
            # ---- load inputs -------------------------------------------
            xq_sb = pp.tile([P, NCT, TQ], bf, name="xq_sb")
            xk_sb = pp.tile([P, NCT, T], bf, name="xk_sb")
            xv_sb = pp.tile([P, NCT, T], bf, name="xv_sb")
            wq_sb = pp.tile([P, NCT, C], bf, name="wq_sb")
            wk_sb = pp.tile([P, NCT, C], bf, name="wk_sb")
            wv_sb = pp.tile([P, NCT, C], bf, name="wv_sb")
            wo_sb = pp.tile([P, NCT, C], bf, name="wo_sb")
            bq_sb = pp.tile([P, NCT], f32, name="bq_sb")
            ev_sb = pp.tile([P, NCT, P], bf, name="ev_sb")
            oh_sb = pp.tile([8, NCT, P], f32, name="oh_sb")
            cor1 = pp.tile([P, H, 5], bf, name="cor1")
            cor2 = pp.tile([P, H, 3], bf, name="cor2")
            am_sb = pp.tile([P, IB, P], bf, name="am_sb")
            for t_sb, t_dr in [(xq_sb, xq), (xk_sb, xk), (xv_sb, xv),
                               (wq_sb, wqT), (wk_sb, wkT), (wv_sb, wvT),
                               (wo_sb, woT), (bq_sb, bq8),
                               (ev_sb, evpr), (oh_sb, oneh), (cor1, cor1d),
                               (cor2, cor2d), (am_sb, amask)]:
                nc.sync.dma_start(t_sb[:], t_dr[:])

            idbf = pp.tile([P, P], bf, name="idbf")
            make_identity(nc, idbf[:])
            idf32 = pp.tile([P, P], f32, name="idf32")
            make_identity(nc, idf32[:])

            # ---- projections -------------------------------------------
            q_sb = pp.tile([P, NCT, TQ], bf, name="q_sb")
            k_sb = pp.tile([P, NCT, T], bf, name="k_sb")
            vT_sb = pp.tile([P, 8, C], bf, name="vT_sb")

            for co in range(NCT):
                pq = psA.tile([P, T], f32, name="mm")[:, 0:TQ]
                for ci in range(NCT):
                    nc.tensor.matmul(pq, wq_sb[:, ci, co * P:(co + 1) * P],
                                     xq_sb[:, ci, :], start=(ci == 0), stop=(ci == 3))
                nc.vector.tensor_scalar_add(q_sb[:, co, :], pq, bq_sb[:, co:co + 1])

            for co in range(NCT):
                pk = psA.tile([P, T], f32, name="mm")
                for ci in range(NCT):
                    for nh in range(2):
                        ns = slice(nh * 512, nh * 512 + 512)
                        nc.tensor.matmul(pk[:, ns],
                                         wk_sb[:, ci, co * P:(co + 1) * P],
                                         xk_sb[:, ci, ns],
                                         start=(ci == 0), stop=(ci == 3))
                nc.vector.tensor_copy(k_sb[:, co, :], pk[:])

            for tb in range(8):
                pv = psA.tile([P, T], f32, name="mm")[:, 0:C]
                for ci in range(NCT):
                    nc.tensor.matmul(pv, xv_sb[:, ci, tb * P:(tb + 1) * P],
                                     wv_sb[:, ci, :], start=(ci == 0), stop=(ci == 3))
                nc.vector.tensor_copy(vT_sb[:, tb, :], pv)

            # ---- scores + exp per (ib, h) ------------------------------
            eT_sb = pp.tile([P, H, 8, IB, P], bf, name="eT_sb")
            rz_sb = pp.tile([8, TQ], f32, name="rz_sb")
            abT_all = pp.tile([P, IB, P], bf, name="abT_all")

            for ib in range(IB):
                e_ib = ep.tile([P, H, T], bf, name="e_ib")
                zall = ep.tile([P, 8], f32, name="zall")
                isl = slice(ib * P, ib * P + P)
                for h in range(H):
                    ct, hp = h // 2, (h % 2) * D
                    sc = psA.tile([P, T], f32, name="mm")
                    for nh in range(2):
                        ns = slice(nh * 512, nh * 512 + 512)
                        nc.tensor.matmul(sc[:, ns],
                                         q_sb[hp:hp + D, ct, isl],
                                         k_sb[hp:hp + D, ct, ns],
                                         start=True, stop=True)
                    if ib == 0:
                        nc.vector.tensor_tensor(out=sc[0:32, 1019:1024],
                                                in0=sc[0:32, 1019:1024],
                                                in1=cor1[0:32, h, :],
                                                op=add_op)
                    if ib == 3:
                        nc.vector.tensor_tensor(out=sc[96:128, 512:515],
                                                in0=sc[96:128, 512:515],
                                                in1=cor2[96:128, h, :],
                                                op=add_op)
                    nc.scalar.activation(e_ib[:, h, :], sc[:], EXP,
                                         accum_out=zall[:, h:h + 1])

                # transpose e (i x j) -> (j x i) per head
                for h in range(H):
                    nc.scalar.dma_start_transpose(
                        eT_sb[:, h, :, ib, :], e_ib[:, h, :])

                # stage band window to DRAM, diagonal readback
                if ib == 0:
                    nc.sync.dma_start(stage[ib, :, :, 4:W], e_ib[:, :, 0:W - 4])
                    nc.sync.dma_start(stage[ib, :, :, 0:4], e_ib[:, :, T - 4:T])
                else:
                    lo = ib * P - 4
                    nc.sync.dma_start(stage[ib], e_ib[:, :, lo:lo + W])

                abg = wp.tile([P, H, 16], bf, name="abg")
                nc.vector.memset(abg[:], 0.0)
                # src flat idx = p*(H*W) + h*W + (p + mt); col = 16*h + mt
                diag = AP(tensor=stage[:].tensor, offset=ib * (P * H * W),
                          ap=[[H * W + 1, P], [W, H], [1, 9]])
                nc.sync.dma_start(abg[:, :, 0:9], diag)
                abm = wp.tile([P, P], bf, name="abm")
                nc.vector.tensor_tensor(
                    out=abm[:], in0=abg[:].rearrange("p h x -> p (h x)"),
                    in1=am_sb[:, ib, :], op=mul_op)
                pabt = psB.tile([P, P], bf, name="tp")
                nc.tensor.transpose(pabt[:], abm[:], idbf[:])
                nc.vector.tensor_copy(abT_all[:, ib, :], pabt[:])

                # Z transpose: (128 x 8) -> (8 x 128)
                pzt = psB.tile([P, P], f32, name="tp")[0:8, :]
                nc.tensor.transpose(pzt, zall[:], idf32[:])
                nc.vector.tensor_copy(rz_sb[:, isl], pzt)

            recz = pp.tile([8, TQ], f32, name="recz")
            nc.vector.reciprocal(recz[:], rz_sb[:])

            # ---- ctx matmuls (per head pair) ---------------------------
            ctxn = pp.tile([P, NCT, TQ], bf, name="ctxn")
            for a in range(NCT):
                pc = psA.tile([P, TQ], f32, name="pc")
                for hh in range(2):
                    h = 2 * a + hh
                    for jb in range(8):
                        nc.tensor.matmul(
                            pc[hh * D:hh * D + D, :],
                            vT_sb[:, jb, h * D:h * D + D],
                            eT_sb[:, h, jb, :, :],
                            start=(jb == 0), stop=False,
                            tile_position=(0, hh * D),
                            skip_group_check=True,
                        )
                # rel_v band term accumulated into the same psum
                for ib in range(IB):
                    nc.tensor.matmul(pc[:, ib * P:(ib + 1) * P],
                                     ev_sb[:, a, :],
                                     abT_all[:, ib, :],
                                     start=False, stop=(ib == IB - 1),
                                     skip_group_check=True)
                # normalize: recipZ broadcast (8 x 512) -> (128 x 512)
                pz = psA.tile([P, TQ], f32, name="pc")
                nc.tensor.matmul(pz[:], oh_sb[:, a, :], recz[:],
                                 start=True, stop=True)
                rzb = wp.tile([P, TQ], f32, name="rzb")
                nc.vector.tensor_copy(rzb[:], pz[:])
                nc.vector.tensor_tensor(out=ctxn[:, a, :], in0=pc[:], in1=rzb[:],
                                        op=mul_op)

            # ---- output projection -------------------------------------
            for co in range(NCT):
                po = psA.tile([P, T], f32, name="mm")[:, 0:TQ]
                for ci in range(NCT):
                    nc.tensor.matmul(po, wo_sb[:, ci, co * P:(co + 1) * P],
                                     ctxn[:, ci, :], start=(ci == 0), stop=(ci == 3))
                o_sb = wp.tile([P, TQ], f32, name="o_sb")
                nc.vector.tensor_copy(o_sb[:], po)
                nc.sync.dma_start(Out[co], o_sb[:])

    nc.compile()
    return nc


def _prep(inputs):
    x_q = np.asarray(inputs["x_q"], np.float32)
    x_k = np.asarray(inputs["x_k"], np.float32)
    x_v = np.asarray(inputs["x_v"], np.float32)
    Wq = np.asarray(inputs["Wq"], np.float32)
    Wk = np.asarray(inputs["Wk"], np.float32)
    Wv = np.asarray(inputs["Wv"], np.float32)
    Wo = np.asarray(inputs["Wo"], np.float32)
    bq = np.asarray(inputs["bq"], np.float32)
    ek = np.asarray(inputs["emb_rel_k"], np.float32)
    ev = np.asarray(inputs["emb_rel_v"], np.float32)

    def ctile(a):  # (C, X) -> (P, NCT, X) partition-first
        return np.ascontiguousarray(a.reshape(NCT, P, -1).transpose(1, 0, 2))

    wqT = ctile(Wq.T * 0.125).astype(BF16)
    wkT = ctile(Wk.T).astype(BF16)
    wvT = ctile(Wv.T).astype(BF16)
    woT = ctile(Wo.T).astype(BF16)
    bq8 = np.ascontiguousarray((bq * 0.125).reshape(NCT, P).T).astype(np.float32)

    evpr = np.zeros((NCT, P, P), np.float32)
    for a in range(NCT):
        for hh in range(2):
            h = 2 * a + hh
            for mt in range(9):
                evpr[a, 32 * a + 16 * hh + mt, hh * D:(hh + 1) * D] = ev[h, mt]
    evpr = np.ascontiguousarray(evpr.transpose(1, 0, 2)).astype(BF16)

    oneh = np.zeros((NCT, 8, P), np.float32)
    for a in range(NCT):
        for cc in range(P):
            oneh[a, 2 * a + cc // D, cc] = 1.0
    oneh = np.ascontiguousarray(oneh.transpose(1, 0, 2))

    # valid-mask for the band gather, per core and i-block
    in_maps = []
    for core in range(8):
        b, half = core // 2, core % 2
        i0 = half * TQ
        # corner score terms, computed on host from a few q columns
        cor1 = np.zeros((P, H, 5), np.float32)
        cor2 = np.zeros((P, H, 3), np.float32)
        if half == 0:
            qc = (Wq @ x_q[b][:, 0:5]) * 0.125 + (bq[:, None] * 0.125)
            for h in range(H):
                for p in range(5):
                    for c in range(p, 5):
                        # j = 1019 + c, i = p, emb row m = c - p
                        cor1[p, h, c] = qc[h * D:(h + 1) * D, p] @ ek[h, c - p]
        else:
            qc = (Wq @ x_q[b][:, 1019:1023]) * 0.125 + (bq[:, None] * 0.125)
            for h in range(H):
                for p in (125, 126, 127):
                    for c in range(3):
                        m = 133 + c - p
                        if 6 <= m <= 8:
                            # i_glob = 896 + p, uses q_{i-1} = col 895 + p
                            cor2[p, h, c] = qc[h * D:(h + 1) * D, (895 + p) - 1019] @ ek[h, m]
        amask = np.zeros((IB, P, P), np.float32)
        for ib in range(IB):
            pvec = np.arange(P)
            for a in range(NCT):
                for mt in range(9):
                    j_g = i0 + ib * P + pvec + mt - 4
                    ok = ((j_g >= 0) & (j_g < T)).astype(np.float32)
                    amask[ib, :, a * 32 + 0 * 16 + mt] = ok
                    amask[ib, :, a * 32 + 1 * 16 + mt] = ok
        amask = np.ascontiguousarray(amask.transpose(1, 0, 2))
        in_maps.append({
            "xq": ctile(x_q[b][:, i0:i0 + TQ]).astype(BF16),
            "xk": ctile(np.roll(x_k[b], -i0, axis=1)).astype(BF16),
            "xv": ctile(np.roll(x_v[b], -i0, axis=1)).astype(BF16),
            "wqT": wqT, "wkT": wkT, "wvT": wvT, "woT": woT,
            "bq8": bq8, "evpr": evpr, "oneh": oneh,
            "cor1d": cor1.astype(BF16), "cor2d": cor2.astype(BF16),
            "amask": amask.astype(BF16),
        })
    return in_maps


def kernel(**inputs):
    if "nc" not in _CACHE:
        _CACHE["nc"] = build_nc()
    nc = _CACHE["nc"]
    in_maps = _prep(inputs)
    res = run_bass_kernel_spmd(nc, in_maps, list(range(8)))
    bo = np.asarray(inputs["bo"], np.float32)
    bv = np.asarray(inputs["bv"], np.float32)
    Wo = np.asarray(inputs["Wo"], np.float32)
    bo_eff = bo + Wo @ bv
    out = np.zeros((4, C, T), np.float32)
    for core in range(8):
        b, half = core // 2, core % 2
        o = np.asarray(res.results[core]["Out"]).reshape(C, TQ)
        out[b][:, half * TQ:(half + 1) * TQ] = o
    out += bo_eff[None, :, None]
    return out

